# revision 8
# baseline (speedup 1.0000x reference)
"""Sparse (sliding-window) attention Trainium2 kernel.

Problem (hardcoded shapes): B=32, N=1024 tokens on a 16x64 grid, C=256,
8 heads, head_dim=32. Local attention window: +-3 grid rows, +-5 grid
cols (7x11). y = softmax(q k^T/sqrt(d) + mask) v, projected.

Sharding: data-parallel over batch, 4 items per core on 8 cores.

Per-core algorithm (bf16 compute, fp32 PSUM accumulation):
  - qkvT[768,1024] = (w_qkv.T).T @ x.T via PE (host passes xT, w_qkv.T
    with the q part pre-scaled by d^-0.5). Layout keeps q/k per head at
    partition offsets 32j, which feeds the row-packed score matmuls.
  - scores in transposed layout ST[k_chunk=128, q_band<=512] per head:
    4 heads computed concurrently via tile_position row packing (K=32).
  - P = exp(ST) on ScalarE (PSUM->SBUF bf16, one op per 4-head group),
    then multiplied by a compact 0/1 band mask on VectorE (2x bf16).
  - out.T[d,q] and denominators accumulate per q-tile in PSUM: PV uses
    col-packed matmuls (lhsT = V chunk [128,32]); the denominator uses
    lhsT = ones [128,32], which lands the row-sum broadcast across the
    32 partitions of each head, so the division is a plain elementwise
    tensor_tensor divide.
  - proj consumes the transposed attention output directly as lhsT.
"""

import numpy as np
import ml_dtypes

import concourse.bass as bass
import concourse.bacc as bacc
import concourse.mybir as mybir
import concourse.tile as tile
from concourse import bass_utils
from concourse.masks import make_identity

F32 = mybir.dt.float32
BF16 = mybir.dt.bfloat16
AF = mybir.ActivationFunctionType

H_MAP, W_MAP = 16, 64
N_TOK = H_MAP * W_MAP            # 1024
DIM = 256
HEADS = 8
HDIM = 32
B_FULL = 32
N_CORES = 8
B_LOC = B_FULL // N_CORES        # 4
NCHUNK = N_TOK // 128            # 8 k-chunks (2 grid rows each)
NQT = N_TOK // 128               # 8 q-tiles


def _qband(c):
    """Valid q range (start token, width) for k-chunk c (rows 2c, 2c+1)."""
    qlo = max(0, 2 * c - 3)
    qhi = min(H_MAP - 1, 2 * c + 4)
    return qlo * W_MAP, (qhi - qlo + 1) * W_MAP


DEBUG_DUMP = False


def build_program():
    nc = bacc.Bacc("TRN2", target_bir_lowering=False, debug=False)

    xt_d = nc.dram_tensor("xt", [B_LOC, DIM, N_TOK], BF16, kind="ExternalInput")
    wqkvT_d = nc.dram_tensor("wqkvT", [DIM, 3 * DIM], BF16, kind="ExternalInput")
    wpT_d = nc.dram_tensor("wpT", [DIM, DIM], BF16, kind="ExternalInput")
    bias_d = nc.dram_tensor("bias", [1, DIM], BF16, kind="ExternalInput")
    maskc_d = nc.dram_tensor("maskc", [NCHUNK, 128, 512], BF16, kind="ExternalInput")
    y_d = nc.dram_tensor("y", [B_LOC, N_TOK, DIM], F32, kind="ExternalOutput")
    if DEBUG_DUMP:
        qkvdbg_d = nc.dram_tensor("qkvdbg", [6, 128, N_TOK], F32, kind="ExternalOutput")
        ptdbg_d = nc.dram_tensor("ptdbg", [NCHUNK, 128, 4, 512], F32, kind="ExternalOutput")
        atdbg_d = nc.dram_tensor("atdbg", [2, 128, N_TOK], F32, kind="ExternalOutput")

    xt = xt_d.ap()
    y = y_d.ap()

    with tile.TileContext(nc) as tc:
        with (
            tc.tile_pool(name="const", bufs=1) as const,
            tc.tile_pool(name="xtp", bufs=4) as xtp,
            tc.tile_pool(name="qkvp", bufs=12) as qkvp,
            tc.tile_pool(name="vp", bufs=20) as vp,
            tc.tile_pool(name="ptp", bufs=7) as ptp,
            tc.tile_pool(name="atp", bufs=4) as atp,
            tc.tile_pool(name="drp", bufs=4) as drp,
            tc.tile_pool(name="yp", bufs=4) as yp,
            tc.tile_pool(name="sc_ps", bufs=1, space="PSUM") as sc_ps,
            tc.tile_pool(name="od_ps", bufs=1, space="PSUM") as od_ps,
            tc.tile_pool(name="mm_ps", bufs=2, space="PSUM") as mm_ps,
        ):
            # ---- constants ----
            wqkv_sb = [const.tile([128, 3 * DIM], BF16, tag=f"wqkv{i}", name=f"wqkv{i}") for i in range(2)]
            for i in range(2):
                nc.sync.dma_start(out=wqkv_sb[i], in_=wqkvT_d.ap()[128 * i:128 * (i + 1), :])
            wp_sb = [const.tile([128, DIM], BF16, tag=f"wp{i}", name=f"wp{i}") for i in range(2)]
            for i in range(2):
                nc.sync.dma_start(out=wp_sb[i], in_=wpT_d.ap()[128 * i:128 * (i + 1), :])
            bias_sb = const.tile([1, DIM], BF16, tag="bias", name="bias_sb")
            nc.sync.dma_start(out=bias_sb, in_=bias_d.ap())
            mask_sb = [const.tile([128, 512], BF16, tag=f"mask{c}", name=f"mask{c}") for c in range(NCHUNK)]
            for c in range(NCHUNK):
                nc.sync.dma_start(out=mask_sb[c], in_=maskc_d.ap()[c])
            ones32 = const.tile([128, 32], BF16, tag="ones32", name="ones32")
            nc.vector.memset(ones32, 1.0)
            ones_row = const.tile([1, 128], BF16, tag="ones_row", name="ones_row")
            nc.vector.memset(ones_row, 1.0)
            ident = const.tile([128, 128], BF16, tag="ident", name="ident")
            make_identity(nc, ident)

            for b in range(B_LOC):
                # ---- qkvT = W @ xT : [768, 1024] as 6 tiles [128, 1024] ----
                xt_sb = [xtp.tile([128, N_TOK], BF16, tag="xt", name="xt_sb") for _ in range(2)]
                for kc in range(2):
                    nc.sync.dma_start(out=xt_sb[kc], in_=xt[b, 128 * kc:128 * (kc + 1), :])
                qkv = [qkvp.tile([128, N_TOK], BF16, tag="qkv", name="qkv_sb") for _ in range(6)]
                for m in range(6):
                    for nh in range(2):
                        ps = mm_ps.tile([128, 512], F32, tag="mm", name="mm_ps_t")
                        for kc in range(2):
                            nc.tensor.matmul(
                                ps,
                                wqkv_sb[kc][:, 128 * m:128 * (m + 1)],
                                xt_sb[kc][:, 512 * nh:512 * (nh + 1)],
                                start=(kc == 0), stop=(kc == 1),
                            )
                        nc.vector.tensor_copy(qkv[m][:, 512 * nh:512 * (nh + 1)], ps)

                if DEBUG_DUMP and b == 0:
                    for m in range(6):
                        dbg = qkvp.tile([128, N_TOK], F32, tag="qdbg", name="qdbg_t", bufs=2)
                        nc.vector.tensor_copy(dbg, qkv[m])
                        nc.sync.dma_start(out=qkvdbg_d.ap()[m], in_=dbg)

                # ---- V tiles per group: [tok 128, 4 heads x 32] ----
                vt = [[vp.tile([128, 128], BF16, tag="v", name="v_sb") for _ in range(NCHUNK)] for _ in range(2)]
                for g in range(2):
                    for t in range(NCHUNK):
                        ps = mm_ps.tile([128, 128], BF16, tag="mm", name="mm_ps_t")
                        nc.tensor.transpose(ps, qkv[4 + g][:, 128 * t:128 * (t + 1)], ident)
                        nc.vector.tensor_copy(vt[g][t], ps)

                aT = [atp.tile([128, N_TOK], BF16, tag="aT", name="aT_sb") for _ in range(2)]
                for g in range(2):
                    pts = [None] * NCHUNK
                    for step in range(NCHUNK + 2):
                        c = step
                        if c < NCHUNK:
                            qs, wc = _qband(c)
                            sc = sc_ps.tile([128, 4, 512], F32, tag="sc", name="sc_t")
                            for j in range(4):
                                nc.tensor.matmul(
                                    sc[:, j, :wc],
                                    qkv[2 + g][32 * j:32 * (j + 1), 128 * c:128 * (c + 1)],
                                    qkv[0 + g][32 * j:32 * (j + 1), qs:qs + wc],
                                    start=True, stop=True,
                                    tile_position=(32 * j, 0),
                                )
                            pt = ptp.tile([128, 4, 512], BF16, tag="pt", name="pt_t")
                            pts[c] = pt
                            nc.scalar.activation(pt[:, :, :wc], sc[:, :, :wc], AF.Exp)
                            # multiply by 0/1 band mask, broadcast across heads
                            m = mask_sb[c][:, :wc]
                            mb = bass.AP(tensor=m.tensor, offset=m.offset,
                                         ap=[m.ap[0], [0, 4], m.ap[1]])
                            nc.vector.tensor_mul(pt[:, :, :wc], pt[:, :, :wc], mb)
                            if DEBUG_DUMP and b == 0 and g == 0:
                                dbg = ptp.tile([128, 4, 512], F32, tag="ptdbg", name="ptdbg_t", bufs=2)
                                nc.vector.memset(dbg, 0.0)
                                nc.vector.tensor_copy(dbg[:, :, :wc], pt[:, :, :wc])
                                nc.sync.dma_start(out=ptdbg_d.ap()[c], in_=dbg)
                        t = step - 2
                        if 0 <= t < NQT:
                            od = od_ps.tile([128, 2, 512], F32, tag="od", name="od_t")
                            cs = [t] + [c2 for c2 in range(t - 2, t + 3)
                                        if 0 <= c2 < NCHUNK and c2 != t]
                            for ci, c2 in enumerate(cs):
                                qs2, wc2 = _qband(c2)
                                lo = max(128 * t, qs2)
                                hi = min(128 * (t + 1), qs2 + wc2)
                                assert lo < hi
                                po = lo - qs2      # offset into chunk band
                                oo = lo - 128 * t  # offset into out tile
                                nw = hi - lo
                                first = ci == 0
                                last = ci == len(cs) - 1
                                for j in range(4):
                                    nc.tensor.matmul(
                                        od[32 * j:32 * (j + 1), 0, oo:oo + nw],
                                        vt[g][c2][:, 32 * j:32 * (j + 1)],
                                        pts[c2][:, j, po:po + nw],
                                        start=first, stop=last,
                                        tile_position=(0, 32 * j),
                                        skip_group_check=True,
                                    )
                                    nc.tensor.matmul(
                                        od[32 * j:32 * (j + 1), 1, oo:oo + nw],
                                        ones32[:, :32],
                                        pts[c2][:, j, po:po + nw],
                                        start=first, stop=last,
                                        tile_position=(0, 32 * j),
                                        skip_group_check=True,
                                    )
                            rc = drp.tile([128, 128], F32, tag="rc", name="rc_t")
                            nc.vector.reciprocal(rc, od[:, 1, 0:128])
                            nc.vector.tensor_mul(
                                aT[g][:, 128 * t:128 * (t + 1)],
                                od[:, 0, 0:128], rc,
                            )

                if DEBUG_DUMP and b == 0:
                    for g in range(2):
                        dbg = atp.tile([128, N_TOK], F32, tag="atdbg", name="atdbg_t", bufs=2)
                        nc.vector.tensor_copy(dbg, aT[g])
                        nc.sync.dma_start(out=atdbg_d.ap()[g], in_=dbg)

                # ---- proj: y[tok,256] = aT.T @ wpT + bias ----
                for t in range(NQT):
                    ps = mm_ps.tile([128, DIM], F32, tag="mm", name="mm_ps_t", padded_shape=[128, 512])
                    for g in range(2):
                        nc.tensor.matmul(
                            ps, aT[g][:, 128 * t:128 * (t + 1)], wp_sb[g],
                            start=(g == 0), stop=False,
                        )
                    nc.tensor.matmul(ps, ones_row, bias_sb, start=False, stop=True)
                    yt = yp.tile([128, DIM], F32, tag="y", name="y_sb")
                    nc.vector.tensor_copy(yt, ps)
                    nc.sync.dma_start(out=y[b, 128 * t:128 * (t + 1), :], in_=yt)

    nc.finalize()
    return nc


_PROGRAM = None


def _get_program():
    global _PROGRAM
    if _PROGRAM is None:
        _PROGRAM = build_program()
    return _PROGRAM


def _prep_inputs(x, w_qkv, w_proj, b_proj, mask):
    """Host-side prep: shard, transpose, cast, compact mask."""
    scale = HDIM ** -0.5
    wqkvT = np.asarray(w_qkv, np.float32).T.copy()       # [256, 768]
    wqkvT[:, :DIM] *= scale                              # fold qk scale into q
    wqkvT = wqkvT.astype(ml_dtypes.bfloat16)
    wpT = np.asarray(w_proj, np.float32).T.astype(ml_dtypes.bfloat16)
    bias = np.asarray(b_proj, np.float32).reshape(1, DIM).astype(ml_dtypes.bfloat16)

    m4 = np.asarray(mask, np.float32).reshape(N_TOK, N_TOK)  # [q, k] additive
    maskc = np.zeros((NCHUNK, 128, 512), np.float32)
    for c in range(NCHUNK):
        qs, wc = _qband(c)
        # rows: k tokens of chunk c; cols: q tokens of the band
        maskc[c, :, :wc] = (m4[qs:qs + wc, 128 * c:128 * (c + 1)] == 0.0).T
    maskc = maskc.astype(ml_dtypes.bfloat16)

    x = np.asarray(x, np.float32)
    in_maps = []
    for core in range(N_CORES):
        xs = x[core * B_LOC:(core + 1) * B_LOC]          # [4, 1024, 256]
        xtl = np.ascontiguousarray(xs.transpose(0, 2, 1)).astype(ml_dtypes.bfloat16)
        in_maps.append({"xt": xtl, "wqkvT": wqkvT, "wpT": wpT,
                        "bias": bias, "maskc": maskc})
    return in_maps


def run(inputs, trace=False):
    nc = _get_program()
    in_maps = _prep_inputs(**inputs)
    res = bass_utils.run_bass_kernel_spmd(
        nc, in_maps, core_ids=list(range(N_CORES)), trace=trace,
    )
    out = np.concatenate([res.results[i]["y"] for i in range(N_CORES)], axis=0)
    return out, res


def kernel(**inputs) -> np.ndarray:
    out, _ = run(inputs, trace=False)
    return out


# revision 11
# speedup vs baseline: 208.2411x; 208.2411x over previous
"""Sparse (sliding-window) attention Trainium2 kernel.

Problem (hardcoded shapes): B=32, N=1024 tokens on a 16x64 grid, C=256,
8 heads, head_dim=32. Local attention window: +-3 grid rows, +-5 grid
cols (7x11). y = softmax(q k^T/sqrt(d) + mask) v, projected.

Sharding: data-parallel over batch, 4 items per core on 8 cores.

Per-core algorithm (bf16 compute, fp32 PSUM accumulation):
  - qkvT[768,1024] = (w_qkv.T).T @ x.T via PE (host passes xT, w_qkv.T
    with the q part pre-scaled by d^-0.5). Layout keeps q/k per head at
    partition offsets 32j, which feeds the row-packed score matmuls.
  - scores in transposed layout ST[k_chunk=128, q_band<=512] per head:
    4 heads computed concurrently via tile_position row packing (K=32).
  - P = exp(ST) on ScalarE (PSUM->SBUF bf16, one op per 4-head group),
    then multiplied by a compact 0/1 band mask on VectorE (2x bf16).
  - out.T[d,q] and denominators accumulate per q-tile in PSUM: PV uses
    col-packed matmuls (lhsT = V chunk [128,32]); the denominator uses
    lhsT = ones [128,32], which lands the row-sum broadcast across the
    32 partitions of each head. PV and denominator live in different
    PSUM banks (start=True clears the whole bank for the written
    partitions, so they must not share one).
  - proj consumes the transposed attention output directly as lhsT.
"""

import contextlib

import numpy as np
import ml_dtypes

import concourse.bass as bass
import concourse.bacc as bacc
import concourse.mybir as mybir
import concourse.tile as tile
from concourse import bass_utils
from concourse.masks import make_identity

F32 = mybir.dt.float32
BF16 = mybir.dt.bfloat16
AF = mybir.ActivationFunctionType

H_MAP, W_MAP = 16, 64
N_TOK = H_MAP * W_MAP            # 1024
DIM = 256
HEADS = 8
HDIM = 32
B_FULL = 32
N_CORES = 8
B_LOC = B_FULL // N_CORES        # 4
NCHUNK = N_TOK // 128            # 8 k-chunks (2 grid rows each)
NQT = N_TOK // 128               # 8 q-tiles


def _qband(c):
    """Valid q range (start token, width) for k-chunk c (rows 2c, 2c+1)."""
    qlo = max(0, 2 * c - 3)
    qhi = min(H_MAP - 1, 2 * c + 4)
    return qlo * W_MAP, (qhi - qlo + 1) * W_MAP


def build_program(loop_n=1):
    nc = bacc.Bacc("TRN2", target_bir_lowering=False, debug=False)

    xt_d = nc.dram_tensor("xt", [B_LOC, DIM, N_TOK], BF16, kind="ExternalInput")
    wqkvT_d = nc.dram_tensor("wqkvT", [DIM, 3 * DIM], BF16, kind="ExternalInput")
    wpT_d = nc.dram_tensor("wpT", [DIM, DIM], BF16, kind="ExternalInput")
    bias_d = nc.dram_tensor("bias", [1, DIM], BF16, kind="ExternalInput")
    maskc_d = nc.dram_tensor("maskc", [NCHUNK, 128, 512], BF16, kind="ExternalInput")
    y_d = nc.dram_tensor("y", [B_LOC, N_TOK, DIM], F32, kind="ExternalOutput")

    xt = xt_d.ap()
    y = y_d.ap()

    with tile.TileContext(nc) as tc:
        with (
            tc.tile_pool(name="const", bufs=1) as const,
            tc.tile_pool(name="xtp", bufs=4) as xtp,
            tc.tile_pool(name="qkvp", bufs=12) as qkvp,
            tc.tile_pool(name="vp", bufs=20) as vp,
            tc.tile_pool(name="ptp", bufs=7) as ptp,
            tc.tile_pool(name="atp", bufs=4) as atp,
            tc.tile_pool(name="drp", bufs=4) as drp,
            tc.tile_pool(name="yp", bufs=4) as yp,
            tc.tile_pool(name="sc_ps", bufs=1, space="PSUM") as sc_ps,
            tc.tile_pool(name="od_ps", bufs=1, space="PSUM") as od_ps,
            tc.tile_pool(name="mm_ps", bufs=2, space="PSUM") as mm_ps,
        ):
            # ---- constants ----
            wqkv_sb = [const.tile([128, 3 * DIM], BF16, tag=f"wqkv{i}", name=f"wqkv{i}")
                       for i in range(2)]
            for i in range(2):
                nc.sync.dma_start(out=wqkv_sb[i], in_=wqkvT_d.ap()[128 * i:128 * (i + 1), :])
            wp_sb = [const.tile([128, DIM], BF16, tag=f"wp{i}", name=f"wp{i}")
                     for i in range(2)]
            for i in range(2):
                nc.sync.dma_start(out=wp_sb[i], in_=wpT_d.ap()[128 * i:128 * (i + 1), :])
            bias_sb = const.tile([1, DIM], BF16, tag="bias", name="bias_sb")
            nc.sync.dma_start(out=bias_sb, in_=bias_d.ap())
            mask_sb = [const.tile([128, 512], BF16, tag=f"mask{c}", name=f"mask{c}")
                       for c in range(NCHUNK)]
            for c in range(NCHUNK):
                nc.sync.dma_start(out=mask_sb[c], in_=maskc_d.ap()[c])
            ones32 = const.tile([128, 32], BF16, tag="ones32", name="ones32")
            nc.vector.memset(ones32, 1.0)
            ones_row = const.tile([1, 128], BF16, tag="ones_row", name="ones_row")
            nc.vector.memset(ones_row, 1.0)
            ident = const.tile([128, 128], BF16, tag="ident", name="ident")
            make_identity(nc, ident)

            loop_cm = tc.For_i(0, loop_n, 1) if loop_n > 1 else contextlib.nullcontext()
            with loop_cm:
                for b in range(B_LOC):
                    # ---- qkvT = W @ xT : [768, 1024] as 6 tiles [128, 1024] ----
                    xt_sb = [xtp.tile([128, N_TOK], BF16, tag="xt", name="xt_sb")
                             for _ in range(2)]
                    for kc in range(2):
                        nc.sync.dma_start(out=xt_sb[kc], in_=xt[b, 128 * kc:128 * (kc + 1), :])
                    qkv = [qkvp.tile([128, N_TOK], BF16, tag="qkv", name="qkv_sb")
                           for _ in range(6)]
                    for m in range(6):
                        for nh in range(2):
                            ps = mm_ps.tile([128, 512], F32, tag="mm", name="mm_ps_t")
                            for kc in range(2):
                                nc.tensor.matmul(
                                    ps,
                                    wqkv_sb[kc][:, 128 * m:128 * (m + 1)],
                                    xt_sb[kc][:, 512 * nh:512 * (nh + 1)],
                                    start=(kc == 0), stop=(kc == 1),
                                )
                            nc.vector.tensor_copy(qkv[m][:, 512 * nh:512 * (nh + 1)], ps)

                    # ---- V tiles per group: [tok 128, 4 heads x 32] ----
                    vt = [[vp.tile([128, 128], BF16, tag="v", name="v_sb")
                           for _ in range(NCHUNK)] for _ in range(2)]
                    for g in range(2):
                        for t in range(NCHUNK):
                            ps = mm_ps.tile([128, 128], BF16, tag="mm", name="mm_ps_t")
                            nc.tensor.transpose(ps, qkv[4 + g][:, 128 * t:128 * (t + 1)], ident)
                            nc.vector.tensor_copy(vt[g][t], ps)

                    aT = [atp.tile([128, N_TOK], BF16, tag="aT", name="aT_sb")
                          for _ in range(2)]
                    for g in range(2):
                        pts = [None] * NCHUNK
                        for step in range(NCHUNK + 2):
                            c = step
                            if c < NCHUNK:
                                qs, wc = _qband(c)
                                sc = sc_ps.tile([128, 4, 512], F32, tag="sc", name="sc_t")
                                for j in range(4):
                                    nc.tensor.matmul(
                                        sc[:, j, :wc],
                                        qkv[2 + g][32 * j:32 * (j + 1), 128 * c:128 * (c + 1)],
                                        qkv[0 + g][32 * j:32 * (j + 1), qs:qs + wc],
                                        start=True, stop=True,
                                        tile_position=(32 * j, 0),
                                    )
                                pt = ptp.tile([128, 4, 512], BF16, tag="pt", name="pt_t")
                                pts[c] = pt
                                nc.scalar.activation(pt[:, :, :wc], sc[:, :, :wc], AF.Exp)
                                # multiply by 0/1 band mask, broadcast across heads
                                m = mask_sb[c][:, :wc]
                                mb = bass.AP(tensor=m.tensor, offset=m.offset,
                                             ap=[m.ap[0], [0, 4], m.ap[1]])
                                nc.vector.tensor_mul(pt[:, :, :wc], pt[:, :, :wc], mb)
                            t = step - 2
                            if 0 <= t < NQT:
                                od = od_ps.tile([128, 2, 512], F32, tag="od", name="od_t")
                                cs = [t] + [c2 for c2 in range(t - 2, t + 3)
                                            if 0 <= c2 < NCHUNK and c2 != t]
                                for ci, c2 in enumerate(cs):
                                    qs2, wc2 = _qband(c2)
                                    lo = max(128 * t, qs2)
                                    hi = min(128 * (t + 1), qs2 + wc2)
                                    po = lo - qs2      # offset into chunk band
                                    oo = lo - 128 * t  # offset into out tile
                                    nw = hi - lo
                                    first = ci == 0
                                    last = ci == len(cs) - 1
                                    for j in range(4):
                                        nc.tensor.matmul(
                                            od[32 * j:32 * (j + 1), 0, oo:oo + nw],
                                            vt[g][c2][:, 32 * j:32 * (j + 1)],
                                            pts[c2][:, j, po:po + nw],
                                            start=first, stop=last,
                                            tile_position=(0, 32 * j),
                                            skip_group_check=True,
                                        )
                                        nc.tensor.matmul(
                                            od[32 * j:32 * (j + 1), 1, oo:oo + nw],
                                            ones32[:, :32],
                                            pts[c2][:, j, po:po + nw],
                                            start=first, stop=last,
                                            tile_position=(0, 32 * j),
                                            skip_group_check=True,
                                        )
                                rc = drp.tile([128, 128], F32, tag="rc", name="rc_t")
                                nc.vector.reciprocal(rc, od[:, 1, 0:128])
                                nc.vector.tensor_mul(
                                    aT[g][:, 128 * t:128 * (t + 1)],
                                    od[:, 0, 0:128], rc,
                                )

                    # ---- proj: y[tok,256] = aT.T @ wpT + bias ----
                    for t in range(NQT):
                        ps = mm_ps.tile([128, DIM], F32, tag="mm", name="mm_ps_t",
                                        padded_shape=[128, 512])
                        for g in range(2):
                            nc.tensor.matmul(
                                ps, aT[g][:, 128 * t:128 * (t + 1)], wp_sb[g],
                                start=(g == 0), stop=False,
                            )
                        nc.tensor.matmul(ps, ones_row, bias_sb, start=False, stop=True)
                        yt = yp.tile([128, DIM], F32, tag="y", name="y_sb")
                        nc.vector.tensor_copy(yt, ps)
                        nc.sync.dma_start(out=y[b, 128 * t:128 * (t + 1), :], in_=yt)

    nc.finalize()
    return nc


_PROGRAM = None


def _get_program():
    global _PROGRAM
    if _PROGRAM is None:
        _PROGRAM = build_program()
    return _PROGRAM


def _prep_inputs(x, w_qkv, w_proj, b_proj, mask):
    """Host-side prep: shard, transpose, cast, compact mask."""
    scale = HDIM ** -0.5
    wqkvT = np.asarray(w_qkv, np.float32).T.copy()       # [256, 768]
    wqkvT[:, :DIM] *= scale                              # fold qk scale into q
    wqkvT = wqkvT.astype(ml_dtypes.bfloat16)
    wpT = np.asarray(w_proj, np.float32).T.astype(ml_dtypes.bfloat16)
    bias = np.asarray(b_proj, np.float32).reshape(1, DIM).astype(ml_dtypes.bfloat16)

    m4 = np.asarray(mask, np.float32).reshape(N_TOK, N_TOK)  # [q, k] additive
    maskc = np.zeros((NCHUNK, 128, 512), np.float32)
    for c in range(NCHUNK):
        qs, wc = _qband(c)
        # rows: k tokens of chunk c; cols: q tokens of the band
        maskc[c, :, :wc] = (m4[qs:qs + wc, 128 * c:128 * (c + 1)] == 0.0).T
    maskc = maskc.astype(ml_dtypes.bfloat16)

    x = np.asarray(x, np.float32)
    in_maps = []
    for core in range(N_CORES):
        xs = x[core * B_LOC:(core + 1) * B_LOC]          # [4, 1024, 256]
        xtl = np.ascontiguousarray(xs.transpose(0, 2, 1)).astype(ml_dtypes.bfloat16)
        in_maps.append({"xt": xtl, "wqkvT": wqkvT, "wpT": wpT,
                        "bias": bias, "maskc": maskc})
    return in_maps


def run(inputs, trace=False):
    nc = _get_program()
    in_maps = _prep_inputs(**inputs)
    res = bass_utils.run_bass_kernel_spmd(
        nc, in_maps, core_ids=list(range(N_CORES)), trace=trace,
    )
    out = np.concatenate([res.results[i]["y"] for i in range(N_CORES)], axis=0)
    return out, res


def kernel(**inputs) -> np.ndarray:
    out, _ = run(inputs, trace=False)
    return out


# revision 13
# speedup vs baseline: 267.6054x; 1.2851x over previous
"""Sparse (sliding-window) attention Trainium2 kernel.

Problem (hardcoded shapes): B=32, N=1024 tokens on a 16x64 grid, C=256,
8 heads, head_dim=32. Local attention window: +-3 grid rows, +-5 grid
cols (7x11). y = softmax(q k^T/sqrt(d) + mask) v, projected.

Sharding: data-parallel over batch, 4 items per core on 8 cores.

Per-core algorithm (bf16 compute, fp32 PSUM accumulation):
  - qkvT[768,1024] = (w_qkv.T).T @ x.T via PE (host passes xT, w_qkv.T
    with the q part pre-scaled by d^-0.5). Layout keeps q/k per head at
    partition offsets 32j, which feeds the row-packed score matmuls.
  - scores in transposed layout ST[k_chunk=128, q_band<=512] per head:
    4 heads computed concurrently via tile_position row packing (K=32).
  - P = exp(ST) on ScalarE (PSUM->SBUF bf16, one op per 4-head group),
    then multiplied by a compact 0/1 band mask on VectorE (2x bf16).
  - out.T[d,q] and denominators accumulate per q-tile in PSUM: PV uses
    col-packed matmuls (lhsT = V chunk [128,32]); the denominator uses
    lhsT = ones [128,32], which lands the row-sum broadcast across the
    32 partitions of each head. PV and denominator live in different
    PSUM banks (start=True clears the whole bank for the written
    partitions, so they must not share one).
  - proj consumes the transposed attention output directly as lhsT.
"""

import contextlib

import numpy as np
import ml_dtypes

import concourse.bass as bass
import concourse.bacc as bacc
import concourse.mybir as mybir
import concourse.tile as tile
from concourse import bass_utils
from concourse.masks import make_identity

F32 = mybir.dt.float32
BF16 = mybir.dt.bfloat16
AF = mybir.ActivationFunctionType

H_MAP, W_MAP = 16, 64
N_TOK = H_MAP * W_MAP            # 1024
DIM = 256
HEADS = 8
HDIM = 32
B_FULL = 32
N_CORES = 8
B_LOC = B_FULL // N_CORES        # 4
NCHUNK = N_TOK // 128            # 8 k-chunks (2 grid rows each)
NQT = N_TOK // 128               # 8 q-tiles
HALF = 512


def _qband(c):
    """Valid q range (start token, width) for k-chunk c (rows 2c, 2c+1)."""
    qlo = max(0, 2 * c - 3)
    qhi = min(H_MAP - 1, 2 * c + 4)
    return qlo * W_MAP, (qhi - qlo + 1) * W_MAP


def build_program(loop_n=1):
    nc = bacc.Bacc("TRN2", target_bir_lowering=False, debug=False)

    xt_d = nc.dram_tensor("xt", [B_LOC, DIM, N_TOK], BF16, kind="ExternalInput")
    wqkvT_d = nc.dram_tensor("wqkvT", [DIM, 3 * DIM], BF16, kind="ExternalInput")
    wpT_d = nc.dram_tensor("wpT", [DIM, DIM], BF16, kind="ExternalInput")
    bias_d = nc.dram_tensor("bias", [1, DIM], BF16, kind="ExternalInput")
    maskc_d = nc.dram_tensor("maskc", [NCHUNK, 128, 512], BF16, kind="ExternalInput")
    y_d = nc.dram_tensor("y", [B_LOC, N_TOK, DIM], F32, kind="ExternalOutput")

    xt = xt_d.ap()
    y = y_d.ap()

    with tile.TileContext(nc) as tc:
        with (
            tc.tile_pool(name="const", bufs=1) as const,
            tc.tile_pool(name="xtp", bufs=4) as xtp,
            tc.tile_pool(name="qkvp", bufs=12) as qkvp,
            tc.tile_pool(name="vp", bufs=20) as vp,
            tc.tile_pool(name="ptp", bufs=9) as ptp,
            tc.tile_pool(name="atp", bufs=4) as atp,
            tc.tile_pool(name="drp", bufs=2) as drp,
            tc.tile_pool(name="yp", bufs=4) as yp,
            tc.tile_pool(name="sc_ps", bufs=1, space="PSUM") as sc_ps,
            tc.tile_pool(name="od_ps", bufs=1, space="PSUM") as od_ps,
            tc.tile_pool(name="mm_ps", bufs=2, space="PSUM") as mm_ps,
        ):
            # ---- constants ----
            wqkv_sb = [const.tile([128, 3 * DIM], BF16, tag=f"wqkv{i}", name=f"wqkv{i}")
                       for i in range(2)]
            for i in range(2):
                nc.sync.dma_start(out=wqkv_sb[i], in_=wqkvT_d.ap()[128 * i:128 * (i + 1), :])
            wp_sb = [const.tile([128, DIM], BF16, tag=f"wp{i}", name=f"wp{i}")
                     for i in range(2)]
            for i in range(2):
                nc.sync.dma_start(out=wp_sb[i], in_=wpT_d.ap()[128 * i:128 * (i + 1), :])
            bias_sb = const.tile([1, DIM], BF16, tag="bias", name="bias_sb")
            nc.sync.dma_start(out=bias_sb, in_=bias_d.ap())
            mask_sb = [const.tile([128, 512], BF16, tag=f"mask{c}", name=f"mask{c}")
                       for c in range(NCHUNK)]
            for c in range(NCHUNK):
                nc.sync.dma_start(out=mask_sb[c], in_=maskc_d.ap()[c])
            ones32 = const.tile([128, 32], BF16, tag="ones32", name="ones32")
            nc.vector.memset(ones32, 1.0)
            ones_row = const.tile([1, 128], BF16, tag="ones_row", name="ones_row")
            nc.vector.memset(ones_row, 1.0)
            ident = const.tile([128, 128], BF16, tag="ident", name="ident")
            make_identity(nc, ident)

            loop_cm = tc.For_i(0, loop_n, 1) if loop_n > 1 else contextlib.nullcontext()
            with loop_cm:
                for b in range(B_LOC):
                    # ---- qkvT = W @ xT : [768, 1024] as 6 tiles [128, 1024] ----
                    xt_sb = [xtp.tile([128, N_TOK], BF16, tag="xt", name="xt_sb")
                             for _ in range(2)]
                    for kc in range(2):
                        nc.sync.dma_start(out=xt_sb[kc], in_=xt[b, 128 * kc:128 * (kc + 1), :])
                    qkv = [qkvp.tile([128, N_TOK], BF16, tag="qkv", name="qkv_sb")
                           for _ in range(6)]
                    for m in range(6):
                        for nh in range(2):
                            ps = mm_ps.tile([128, 512], F32, tag="mm", name="mm_ps_t")
                            for kc in range(2):
                                nc.tensor.matmul(
                                    ps,
                                    wqkv_sb[kc][:, 128 * m:128 * (m + 1)],
                                    xt_sb[kc][:, 512 * nh:512 * (nh + 1)],
                                    start=(kc == 0), stop=(kc == 1),
                                )
                            nc.vector.tensor_copy(qkv[m][:, 512 * nh:512 * (nh + 1)], ps)

                    # ---- V tiles per group: [tok 128, 4 heads x 32] ----
                    vt = [[vp.tile([128, 128], BF16, tag="v", name="v_sb")
                           for _ in range(NCHUNK)] for _ in range(2)]
                    for g in range(2):
                        for t in range(NCHUNK):
                            ps = mm_ps.tile([128, 128], BF16, tag="mm", name="mm_ps_t")
                            nc.tensor.transpose(ps, qkv[4 + g][:, 128 * t:128 * (t + 1)], ident)
                            nc.vector.tensor_copy(vt[g][t], ps)

                    aT = [atp.tile([128, N_TOK], BF16, tag="aT", name="aT_sb")
                          for _ in range(2)]
                    for g in range(2):
                        pts = [None] * NCHUNK

                        def produce(c, g=g, pts=pts):
                            qs, wc = _qband(c)
                            sc = sc_ps.tile([128, 4, 512], F32, tag="sc", name="sc_t")
                            for j in range(4):
                                nc.tensor.matmul(
                                    sc[:, j, :wc],
                                    qkv[2 + g][32 * j:32 * (j + 1), 128 * c:128 * (c + 1)],
                                    qkv[0 + g][32 * j:32 * (j + 1), qs:qs + wc],
                                    start=True, stop=True,
                                    tile_position=(32 * j, 0),
                                )
                            pt = ptp.tile([128, 4, 512], BF16, tag="pt", name="pt_t")
                            pts[c] = pt
                            nc.scalar.activation(pt[:, :, :wc], sc[:, :, :wc], AF.Exp)
                            # multiply by 0/1 band mask, broadcast across heads
                            m = mask_sb[c][:, :wc]
                            mb = bass.AP(tensor=m.tensor, offset=m.offset,
                                         ap=[m.ap[0], [0, 4], m.ap[1]])
                            nc.vector.tensor_mul(pt[:, :, :wc], pt[:, :, :wc], mb)

                        # q-halves: accumulate out.T/denominator over chunks in
                        # a zeroed 2-bank PSUM tile (start=False throughout —
                        # correct after memset regardless of has_written)
                        for half in (0, 1):
                            h0 = HALF * half
                            prod = [c for c in range(NCHUNK)
                                    if (c <= 5 if half == 0 else c >= 6)]
                            cons = [c for c in range(NCHUNK)
                                    if (c <= 5 if half == 0 else c >= 2)]
                            od = od_ps.tile([128, 2, HALF], F32, tag="od", name="od_t")
                            nc.vector.memset(od, 0.0)
                            for c in prod:
                                produce(c)
                            for ci, c in enumerate(cons):
                                qs, wc = _qband(c)
                                lo = max(h0, qs)
                                hi = min(h0 + HALF, qs + wc)
                                po, oo, nw = lo - qs, lo - h0, hi - lo
                                last = ci == len(cons) - 1
                                for j in range(4):
                                    nc.tensor.matmul(
                                        od[32 * j:32 * (j + 1), 0, oo:oo + nw],
                                        vt[g][c][:, 32 * j:32 * (j + 1)],
                                        pts[c][:, j, po:po + nw],
                                        start=False, stop=last,
                                        tile_position=(0, 32 * j),
                                        skip_group_check=True,
                                    )
                                    nc.tensor.matmul(
                                        od[32 * j:32 * (j + 1), 1, oo:oo + nw],
                                        ones32[:, :32],
                                        pts[c][:, j, po:po + nw],
                                        start=False, stop=last,
                                        tile_position=(0, 32 * j),
                                        skip_group_check=True,
                                    )
                            rc = drp.tile([128, HALF], F32, tag="rc", name="rc_t")
                            nc.vector.reciprocal(rc, od[:, 1, :])
                            nc.vector.tensor_mul(
                                aT[g][:, h0:h0 + HALF], od[:, 0, :], rc)

                    # ---- proj: y[tok,256] = aT.T @ wpT + bias ----
                    for t in range(NQT):
                        ps = mm_ps.tile([128, DIM], F32, tag="mm", name="mm_ps_t",
                                        padded_shape=[128, 512])
                        for g in range(2):
                            nc.tensor.matmul(
                                ps, aT[g][:, 128 * t:128 * (t + 1)], wp_sb[g],
                                start=(g == 0), stop=False,
                            )
                        nc.tensor.matmul(ps, ones_row, bias_sb, start=False, stop=True)
                        yt = yp.tile([128, DIM], F32, tag="y", name="y_sb")
                        nc.vector.tensor_copy(yt, ps)
                        nc.sync.dma_start(out=y[b, 128 * t:128 * (t + 1), :], in_=yt)

    nc.finalize()
    return nc


_PROGRAM = None


def _get_program():
    global _PROGRAM
    if _PROGRAM is None:
        _PROGRAM = build_program()
    return _PROGRAM


def _prep_inputs(x, w_qkv, w_proj, b_proj, mask):
    """Host-side prep: shard, transpose, cast, compact mask."""
    scale = HDIM ** -0.5
    wqkvT = np.asarray(w_qkv, np.float32).T.copy()       # [256, 768]
    wqkvT[:, :DIM] *= scale                              # fold qk scale into q
    wqkvT = wqkvT.astype(ml_dtypes.bfloat16)
    wpT = np.asarray(w_proj, np.float32).T.astype(ml_dtypes.bfloat16)
    bias = np.asarray(b_proj, np.float32).reshape(1, DIM).astype(ml_dtypes.bfloat16)

    m4 = np.asarray(mask, np.float32).reshape(N_TOK, N_TOK)  # [q, k] additive
    maskc = np.zeros((NCHUNK, 128, 512), np.float32)
    for c in range(NCHUNK):
        qs, wc = _qband(c)
        # rows: k tokens of chunk c; cols: q tokens of the band
        maskc[c, :, :wc] = (m4[qs:qs + wc, 128 * c:128 * (c + 1)] == 0.0).T
    maskc = maskc.astype(ml_dtypes.bfloat16)

    x = np.asarray(x, np.float32)
    in_maps = []
    for core in range(N_CORES):
        xs = x[core * B_LOC:(core + 1) * B_LOC]          # [4, 1024, 256]
        xtl = np.ascontiguousarray(xs.transpose(0, 2, 1)).astype(ml_dtypes.bfloat16)
        in_maps.append({"xt": xtl, "wqkvT": wqkvT, "wpT": wpT,
                        "bias": bias, "maskc": maskc})
    return in_maps


def run(inputs, trace=False):
    nc = _get_program()
    in_maps = _prep_inputs(**inputs)
    res = bass_utils.run_bass_kernel_spmd(
        nc, in_maps, core_ids=list(range(N_CORES)), trace=trace,
    )
    out = np.concatenate([res.results[i]["y"] for i in range(N_CORES)], axis=0)
    return out, res


def kernel(**inputs) -> np.ndarray:
    out, _ = run(inputs, trace=False)
    return out
